# revision 1
# baseline (speedup 1.0000x reference)
"""3-layer GCN (PyG GCNConv semantics) on 8 Trainium2 NeuronCores.

Math (per layer, matching the reference exactly):
    y = x @ W
    deg[d] = (# edges into d) + 1,  dinv = deg^-1/2
    out[d] = dinv[d] * ( sum_{e: dst=d} dinv[src_e] * y[src_e] + dinv[d]*y[d] ) + b
The norm dinv[src]*dinv[dst] is separable: fold dinv[src] into a per-node
table  ytab = dinv * (x @ W)  and dinv[dst] into the per-edge mask weight.
The self-loop term is just an extra edge (d, d).

Distribution: nodes (dsts) sharded across 8 cores; each core owns a
contiguous 12544-padded shard.  Each layer:
  1. table phase: core computes ytab for its shard (x_fm slice @ W, scaled),
     writes fp16 128B rows to DRAM, AllGather -> full table replica per core.
  2. aggregation: dma_gather with a PAIR view of the table (elem=128 fp16 =
     256B covering nodes 2k,2k+1; idx = src//2) streams per-edge rows into
     SBUF; per 128-cell column a selection mask (iota == dst_rel) * dinv[dst]
     routes cells to dst columns; PE matmuls (lhsT column half = src parity)
     accumulate a [64, 128] feature-major psum per dst tile.
Cells are laid out by (block, dst-tile, src-parity) segments padded to the
cross-core envelope (max over cores; node->slot assignment is balanced across
cores to keep the envelope near the mean); every 128-cell buffer column gets
per-segment mask columns so matmuls are full-range.  All cores run ONE shared
instruction schedule; per-core structure lives in data tensors (gather idxs,
dst_rel, edinv).
"""

import os
import numpy as np

import concourse.bacc as bacc
import concourse.mybir as mybir
import concourse.tile as tile
from concourse.bass_utils import run_bass_kernel_spmd

P = 128
H = 64
NCORES = 8
GRAN = 1                  # envelope granularity (calls are 128-aligned anyway)
F32 = mybir.dt.float32
F16 = mybir.dt.float16


def _round_up(a, b):
    return (a + b - 1) // b * b


def _balanced_relabel(src, dst, N, shard_real, shard_p, degree):
    """Assign each node a position in its shard (tile = pos//128, parity =
    pos%2) so per-(core, src-block, dst-tile, src-parity) segment counts are
    balanced across cores.  Pass A packs dst nodes into tiles per core
    (balancing in-edge channel sums); pass B picks src parities (balancing
    the even/odd split of every segment globally).  Fallback: degree sort."""
    ntiles = shard_p // P
    nsh_blk = 4                                   # shards per src block
    nblocks = NCORES // nsh_blk
    core_of = dst // shard_real
    src_core = src // shard_real
    src_blk = src_core // nsh_blk

    if os.environ.get("GNN_BAL", "1") != "1":
        pos_list, newpos = [], np.empty(N, np.int64)
        for c in range(NCORES):
            lo, hi = c * shard_real, min(N, (c + 1) * shard_real)
            r = np.argsort(np.argsort(-degree[lo:hi], kind="stable"))
            pos_list.append(r)
            newpos[lo:hi] = r
        return pos_list, newpos

    rng = np.random.default_rng(0)
    # initial parity guess: alternate by degree rank within shard
    parity = np.empty(N, np.int8)
    for c in range(NCORES):
        lo, hi = c * shard_real, min(N, (c + 1) * shard_real)
        r = np.argsort(np.argsort(-degree[lo:hi], kind="stable"))
        parity[lo:hi] = (r % 2).astype(np.int8)

    # edges grouped by src for pass B
    src_order = np.argsort(src, kind="stable")
    src_sorted = src[src_order]
    src_starts = np.searchsorted(src_sorted, np.arange(N + 1))

    tile_of = np.empty(N, np.int64)
    for _ in range(2):
        # ---- pass A: per core, pack dst nodes into tiles (LPT greedy on
        # 4 channels = src_blk * 2 + src_parity, then swap refinement)
        ch = (src_blk * 2 + parity[src]).astype(np.int64)
        for c in range(NCORES):
            lo, hi = c * shard_real, min(N, (c + 1) * shard_real)
            m = core_of == c
            d_loc = dst[m] - lo
            nloc = hi - lo
            node_ch = np.zeros((nloc, 4), np.int64)
            np.add.at(node_ch, (d_loc, ch[m]), 1)
            tot = node_ch.sum(1)
            order = np.argsort(-tot, kind="stable")
            tsum = np.zeros((ntiles, 4), np.float64)
            cap = np.full(ntiles, P, np.int64)
            cap[-1] = P - (shard_p - nloc)         # pads live in last tile
            tgt = node_ch.sum(0).astype(np.float64) / ntiles
            tloc = np.empty(nloc, np.int64)
            for i in order:
                v = node_ch[i].astype(np.float64)
                score = ((tsum + v - tgt) ** 2).sum(1)
                score[cap <= 0] = np.inf
                t = int(np.argmin(score))
                tloc[i] = t
                tsum[t] += v
                cap[t] -= 1
            # swap refinement: move a node from the worst tile-channel to
            # the most complementary tile, swapping with its best partner
            nch = node_ch.astype(np.float64)
            for _r in range(4 * ntiles):
                dev = tsum - tgt
                t_hi = int(np.argmax(np.abs(dev).sum(1)))
                ids_hi = np.nonzero(tloc == t_hi)[0]
                t_lo = int(np.argmin((dev * dev[t_hi]).sum(1)))
                if t_lo == t_hi:
                    break
                ids_lo = np.nonzero(tloc == t_lo)[0]
                # best pair (i from hi, j from lo) minimizing new deviation
                di = nch[ids_hi]
                dj = nch[ids_lo]
                best = None
                cur_cost = (dev[t_hi] ** 2).sum() + (dev[t_lo] ** 2).sum()
                # evaluate a subsample for speed
                ii = ids_hi[np.argsort(-(di * dev[t_hi]).sum(1))[:16]]
                jj = ids_lo[np.argsort((dj * dev[t_hi]).sum(1))[:16]]
                for i_ in ii:
                    d_new_i = dev[t_hi] - nch[i_]
                    for j_ in jj:
                        dh = d_new_i + nch[j_]
                        dl = dev[t_lo] - nch[j_] + nch[i_]
                        cost = (dh ** 2).sum() + (dl ** 2).sum()
                        if best is None or cost < best[0]:
                            best = (cost, i_, j_)
                if best is None or best[0] >= cur_cost - 1e-9:
                    break
                _, i_, j_ = best
                tsum[t_hi] += nch[j_] - nch[i_]
                tsum[t_lo] += nch[i_] - nch[j_]
                tloc[i_], tloc[j_] = t_lo, t_hi
            tile_of[lo:hi] = tloc

        # ---- pass B: global greedy parity choice per src node (imbalance
        # tracked per (src-block, core, tile) segment; a src's block is fixed)
        imb = np.zeros((nblocks, NCORES * ntiles), np.float64)
        bins_dst = core_of * ntiles + tile_of[dst]    # seg bin of each edge
        ubins, umcount = {}, {}
        for u in rng.permutation(N):
            e0, e1 = src_starts[u], src_starts[u + 1]
            if e0 == e1:
                continue
            b, mcount = np.unique(bins_dst[src_order[e0:e1]],
                                  return_counts=True)
            ubins[u], umcount[u] = b, mcount
            bu = (u // shard_real) // nsh_blk
            cur = imb[bu, b]
            p_ = 0 if ((cur + mcount) ** 2).sum() <= ((cur - mcount) ** 2).sum() else 1
            parity[u] = p_
            imb[bu, b] += mcount if p_ == 0 else -mcount
        for _s in range(3):                        # refinement resweeps
            for u in ubins:
                b, mcount = ubins[u], umcount[u]
                bu = (u // shard_real) // nsh_blk
                sgn = 1 if parity[u] == 0 else -1
                cur = imb[bu, b] - sgn * mcount    # without u
                p_ = 0 if ((cur + mcount) ** 2).sum() <= ((cur - mcount) ** 2).sum() else 1
                if p_ != parity[u]:
                    parity[u] = p_
                    imb[bu, b] = cur + (mcount if p_ == 0 else -mcount)

    # ---- final slots: within each (core, tile), interleave by parity with
    # capacity 64/64 (overflow nodes take whatever slot class is left)
    pos_list, newpos = [], np.empty(N, np.int64)
    for c in range(NCORES):
        lo, hi = c * shard_real, min(N, (c + 1) * shard_real)
        nloc = hi - lo
        pos = np.empty(nloc, np.int64)
        for t in range(ntiles):
            ids = np.nonzero(tile_of[lo:hi] == t)[0]
            ev = ids[parity[lo + ids] == 0]
            od = ids[parity[lo + ids] == 1]
            if len(ev) > 64:
                od = np.concatenate([od, ev[64:]]); ev = ev[:64]
            if len(od) > 64:
                ev = np.concatenate([ev, od[64:]]); od = od[:64]
            pos[ev] = t * P + 2 * np.arange(len(ev))
            pos[od] = t * P + 2 * np.arange(len(od)) + 1
        pos_list.append(pos)
        newpos[lo:hi] = pos
    return pos_list, newpos


# ----------------------------------------------------------------------------
# host-side schedule + per-core tensors
# ----------------------------------------------------------------------------
def _build_schedule(edge_index, N):
    src = np.asarray(edge_index[0], dtype=np.int64)
    dst = np.asarray(edge_index[1], dtype=np.int64)

    shard_real = (N + NCORES - 1) // NCORES           # 12500
    shard_p = _round_up(shard_real, P)                # 12544
    ntiles = shard_p // P                             # 98
    blk_nodes = 4 * shard_p                           # 50176 (25088 pairs < 2^15)
    nblocks = (NCORES * shard_p + blk_nodes - 1) // blk_nodes  # 2

    degree = np.bincount(dst, minlength=N).astype(np.int64)
    deg = degree.astype(np.float64) + 1.0
    dinv = (1.0 / np.sqrt(deg)).astype(np.float32)

    # Relabeling per shard: pick each node's (tile, even/odd slot) to balance
    # the per-(core, block, tile, parity) segment counts across cores, so the
    # cross-core envelope (max over cores) stays near the mean.
    pos_list, newpos = _balanced_relabel(src, dst, N, shard_real, shard_p, degree)

    core_of = dst // shard_real
    src_gid = (src // shard_real) * shard_p + newpos[src]
    ldst = newpos[dst]

    blk = src_gid // blk_nodes
    pair_loc = (src_gid % blk_nodes) // 2
    par = src_gid % 2
    tile_id = ldst // P
    seg = (blk * ntiles + tile_id) * 2 + par          # (b, t, par) segment
    nseg = nblocks * ntiles * 2

    counts = np.zeros((NCORES, nseg), np.int64)
    per_core = []
    for c in range(NCORES):
        m = core_of == c
        s_c = seg[m]
        counts[c] = np.bincount(s_c, minlength=nseg)
        per_core.append((s_c, pair_loc[m], ldst[m], dst[m]))

    smax = counts.max(0)
    S_seg = _round_up(smax, GRAN)                     # padded cells per segment

    def seg_id(b, t, p_):
        return (b * ntiles + t) * 2 + p_

    def seg_sum(b, ta, tb_):
        return int(S_seg[seg_id(b, ta, 0):seg_id(b, tb_ - 1, 1) + 1].sum())

    # ---- tile groups: per (group, block) one gather call of <= MCB columns;
    # boundaries chosen by DP to minimize the 128-alignment padding of calls
    MCB = int(os.environ.get("GNN_MCB", "28"))
    pref = np.zeros((nblocks, ntiles + 1), np.int64)
    for b in range(nblocks):
        for t in range(ntiles):
            pref[b, t + 1] = pref[b, t] + S_seg[seg_id(b, t, 0)] + \
                S_seg[seg_id(b, t, 1)]
    LAM = 32                                      # small bias to fewer calls
    best = np.full(ntiles + 1, 1 << 60, np.int64)
    prev = np.full(ntiles + 1, -1, np.int64)
    best[0] = 0
    for t1 in range(1, ntiles + 1):
        for t0 in range(t1 - 1, -1, -1):
            ss = pref[:, t1] - pref[:, t0]
            if (-(-ss // P)).max() > MCB:
                break
            pad = int(((-ss) % P).sum()) + LAM
            if best[t0] + pad < best[t1]:
                best[t1] = best[t0] + pad
                prev[t1] = t0
    tile_groups = []
    t1 = ntiles
    while t1 > 0:
        t0 = int(prev[t1])
        tile_groups.append((t0, t1))
        t1 = t0
    tile_groups.reverse()

    # ---- cell layout: calls laid out in (group, block) order, each call
    # 128-aligned so the gather writes every row any matmul reads.
    base = np.zeros(nseg, np.int64)
    calls = []          # per group: list over blocks of (cell0, ncells)
    cur = 0
    for (t0, t1) in tile_groups:
        cinfo = []
        for b in range(nblocks):
            cell0 = cur
            for t in range(t0, t1):
                for p_ in range(2):
                    s = seg_id(b, t, p_)
                    base[s] = cur
                    cur += int(S_seg[s])
            cur = _round_up(cur, P)
            cinfo.append((cell0, cur - cell0))
        calls.append(cinfo)
    total_cells = cur

    # ---- static piece table: one full-range matmul per (segment, buf column)
    # with a dedicated mask column (rows outside the segment span disabled).
    group_pieces = []   # per group: {t: [(b, mask_col, buf_col, parity)]}
    mc_base = np.zeros(nseg, np.int64)   # first mask col of each segment
    mc = 0
    for gidx, (t0, t1) in enumerate(tile_groups):
        cinfo = calls[gidx]
        pieces = {}
        for t in range(t0, t1):
            pl = []
            for b in range(nblocks):
                c0 = cinfo[b][0]
                for p_ in range(2):
                    s = seg_id(b, t, p_)
                    a0 = int(base[s]) - c0
                    a1 = a0 + int(S_seg[s])
                    mc_base[s] = mc
                    for col in range(a0 // P, -(-a1 // P)):
                        pl.append((b, mc, col, p_))
                        mc += 1
            pieces[t] = pl
        group_pieces.append(pieces)
    ncols_total = mc

    # ---- per-core cell tensors
    core_tensors = []
    # per-cell mapping: global pos -> call-local (partition, buf col, mask col)
    call_cell0 = np.sort(np.array(
        [cinfo[b][0] for cinfo in calls for b in range(nblocks)], np.int64))

    def cell_to_rowcol(pos, s_s):
        ci = np.searchsorted(call_cell0, pos, side="right") - 1
        local = pos - call_cell0[ci]
        col = local // P
        a0col = (base[s_s] - call_cell0[ci]) // P
        return (local % P).astype(np.int64), mc_base[s_s] + col - a0col

    for c in range(NCORES):
        s_c, pl_c, ld_c, d_c = per_core[c]
        order = np.argsort(s_c, kind="stable")
        s_s = s_c[order]
        cnt = counts[c]
        starts = np.zeros(nseg, np.int64)
        np.cumsum(cnt[:-1], out=starts[1:])
        rank = np.arange(s_s.shape[0], dtype=np.int64) - np.repeat(starts, cnt)
        pos = base[s_s] + rank
        prow, mcol = cell_to_rowcol(pos, s_s)

        idx_cells = np.zeros(total_cells, np.int16)          # pad -> pair 0
        dstrel = np.full((P, ncols_total + ntiles), -1.0, np.float32)
        edinv = np.zeros((P, ncols_total + ntiles), np.float32)
        idx_cells[pos] = pl_c[order].astype(np.int16)
        dstrel[prow, mcol] = (ld_c[order] % P).astype(np.float32)
        edinv[prow, mcol] = dinv[d_c[order]].astype(np.float32)

        idx16 = idx_cells.reshape(-1, 16).T                  # [16, cells/16]
        idx_w = np.tile(idx16, (8, 1)).copy()                # [128, cells/16]

        lo = c * shard_real
        hi = min(N, lo + shard_real)
        dvec = np.ones(shard_p, np.float32)
        dvec[pos_list[c]] = dinv[lo:hi]
        dinv_col = dvec.reshape(ntiles, P).T.astype(np.float32).copy()

        # self columns (one per tile, after gather cols)
        valid = np.zeros(shard_p, bool)
        valid[pos_list[c]] = True
        sd = np.full(shard_p, -1.0, np.float32)
        se = np.zeros(shard_p, np.float32)
        sd[valid] = (np.arange(shard_p) % P).astype(np.float32)[valid]
        se[valid] = dvec[valid]
        dstrel[:, ncols_total:] = sd.reshape(ntiles, P).T
        edinv[:, ncols_total:] = se.reshape(ntiles, P).T

        core_tensors.append(
            dict(idx=idx_w, dst_rel=np.ascontiguousarray(dstrel),
                 edinv=np.ascontiguousarray(edinv), dinv_col=dinv_col,
                 pos=pos_list[c])
        )

    sched = dict(
        shard_real=shard_real,
        shard_p=shard_p,
        blk_nodes=blk_nodes,
        nblocks=nblocks,
        ntiles=ntiles,
        tile_groups=tile_groups,
        calls=calls,
        group_pieces=group_pieces,
        ncols_total=ncols_total,
        total_cells=total_cells,
    )
    return sched, core_tensors


# ----------------------------------------------------------------------------
# bass builder
# ----------------------------------------------------------------------------
def _build_bass(sched):
    shard_p = sched["shard_p"]
    blk_nodes = sched["blk_nodes"]
    nblocks = sched["nblocks"]
    ntiles = sched["ntiles"]
    tile_groups = sched["tile_groups"]
    calls = sched["calls"]
    group_pieces = sched["group_pieces"]
    ncols_total = sched["ncols_total"]
    total_cells = sched["total_cells"]
    table_rows = NCORES * shard_p
    blk_pairs = blk_nodes // 2

    TSIM = bool(int(os.environ.get("GNN_TSIM", "0")))
    nc = bacc.Bacc("TRN2", target_bir_lowering=False,
                   num_devices=1 if TSIM else NCORES,
                   dynamic_dma_scratch_size=int(os.environ.get("GNN_SCRATCH", "65536")))

    emb_in = nc.dram_tensor("emb_fm", [H, shard_p], F16, kind="ExternalInput")
    idx_in = nc.dram_tensor("idx", [P, total_cells // 16], mybir.dt.int16,
                            kind="ExternalInput")
    dst_rel_in = nc.dram_tensor("dst_rel", [P, ncols_total + ntiles], F32,
                                kind="ExternalInput")
    edinv_in = nc.dram_tensor("edinv", [P, ncols_total + ntiles], F32,
                              kind="ExternalInput")
    dinv_col_in = nc.dram_tensor("dinv_col", [P, ntiles], F32, kind="ExternalInput")
    iota_in = nc.dram_tensor("iota", [P, P], F16, kind="ExternalInput")
    w_ins = [nc.dram_tensor(f"W{l+1}", [H, H], F16, kind="ExternalInput")
             for l in range(3)]
    b_ins = [nc.dram_tensor(f"b{l+1}", [H, 1], F32, kind="ExternalInput")
             for l in range(3)]
    out_fm = nc.dram_tensor("out_fm", [H, shard_p], F16, kind="ExternalOutput")

    NL = int(os.environ.get("GNN_NLAYERS", "3"))
    ACT_EVAC = os.environ.get("GNN_ACT", "0") == "1"
    MGB = int(os.environ.get("GNN_MGB", "5"))

    with tile.TileContext(nc) as tc:
        with (
            tc.tile_pool(name="persist", bufs=1) as persist,
            tc.tile_pool(name="msgs", bufs=MGB) as msgs_pool,
            tc.tile_pool(name="masks", bufs=int(os.environ.get("GNN_MKB", "16"))) as mask_pool,
            tc.tile_pool(name="ps_agg", bufs=int(os.environ.get("GNN_PSB", "6")), space="PSUM") as ps_agg,
            tc.tile_pool(name="ps_tb", bufs=2, space="PSUM") as ps_tb,
            tc.tile_pool(name="dram", bufs=1, space="DRAM") as dram,
        ):
            # ---- persistent SBUF ----
            x_fm = persist.tile([H, shard_p], F16)
            yshard = persist.tile([P, ntiles, H], F16)
            idx_sb = persist.tile([P, total_cells // 16], mybir.dt.int16)
            dst_rel = persist.tile([P, ncols_total + ntiles], F32)
            edinv = persist.tile([P, ncols_total + ntiles], F32)
            dinv_col = persist.tile([P, ntiles], F32)
            iota_sb = persist.tile([P, P], F16)
            w_sb = [persist.tile([H, H], F16, name=f"w{l}") for l in range(3)]
            b_sb = [persist.tile([H, 1], F32, name=f"b{l}") for l in range(3)]

            nc.sync.dma_start(out=x_fm[:], in_=emb_in[:])
            nc.sync.dma_start(out=idx_sb[:], in_=idx_in[:])
            nc.sync.dma_start(out=dst_rel[:], in_=dst_rel_in[:])
            nc.sync.dma_start(out=edinv[:], in_=edinv_in[:])
            nc.sync.dma_start(out=dinv_col[:], in_=dinv_col_in[:])
            nc.sync.dma_start(out=iota_sb[:], in_=iota_in[:])
            for l in range(3):
                nc.sync.dma_start(out=w_sb[l][:], in_=w_ins[l][:])
                nc.sync.dma_start(out=b_sb[l][:], in_=b_ins[l][:])

            ag_in = [dram.tile([shard_p, H], F16, name=f"agin{i}") for i in range(3)]
            tables = [dram.tile([table_rows, H], F16, addr_space="Shared",
                                name=f"table{i}") for i in range(3)]

            def mk_mask(col):
                mask = mask_pool.tile([P, P], F16, tag="mask")
                nc.vector.tensor_scalar(
                    mask[:], iota_sb[:],
                    dst_rel[:, col:col + 1],
                    edinv[:, col:col + 1],
                    mybir.AluOpType.is_equal,
                    mybir.AluOpType.mult,
                )
                return mask

            def table_tile(l, t):
                pt = ps_tb.tile([P, H], F32, space="PSUM", tag="pt")
                nc.tensor.matmul(
                    out=pt[:],
                    lhsT=x_fm[:, t * P:(t + 1) * P],
                    rhs=w_sb[l][:],
                    start=True, stop=True,
                )
                if ACT_EVAC:
                    nc.scalar.mul(yshard[:, t, :], pt[:], dinv_col[:, t:t + 1])
                else:
                    nc.vector.tensor_scalar_mul(
                        yshard[:, t, :], pt[:], dinv_col[:, t:t + 1])

            def gi_write(l, t0, t1):
                nc.sync.dma_start(
                    out=ag_in[l][t0 * P:t1 * P, :].rearrange(
                        "(a p) h -> p a h", p=P),
                    in_=yshard[:, t0:t1, :],
                )

            def distribute(l):
                if TSIM:
                    nc.sync.dma_start(out=tables[l][0:shard_p, :],
                                      in_=ag_in[l][:])
                else:
                    nc.gpsimd.collective_compute(
                        "AllGather",
                        mybir.AluOpType.bypass,
                        replica_groups=[list(range(NCORES))],
                        ins=[ag_in[l][:].opt()],
                        outs=[tables[l][: NCORES * shard_p, :].opt()],
                    )

            # ---- prologue: layer-0 table from the embeddings ----
            for t in range(ntiles):
                table_tile(0, t)
            for t0 in range(0, ntiles, 14):
                gi_write(0, t0, min(ntiles, t0 + 14))
            distribute(0)

            # emit groups largest-first so the smallest group drains the
            # pipeline at each layer boundary (shortest AllGather stall)
            emit_order = sorted(
                range(len(tile_groups)),
                key=lambda g: -max(c[1] for c in calls[g]))
            for l in range(NL):
                tb = tables[l]
                # ---- aggregation (tile groups; pair-view gather); the next
                # layer's table phase is interleaved per tile so only the
                # AllGather remains at the layer boundary ----
                for gidx in emit_order:
                    t0, t1 = tile_groups[gidx]
                    cinfo = calls[gidx]
                    bufs = []
                    for b in range(nblocks):
                        cell0, ncells = cinfo[b]
                        if ncells == 0:
                            bufs.append(None)
                            continue
                        ncols = -(-ncells // P)
                        buf = msgs_pool.tile([P, ncols, 2 * H], F16,
                                             tag=f"msgs{b}", bufs=MGB)
                        nc.gpsimd.dma_gather(
                            buf[:],
                            tb[b * blk_nodes:
                               min((b + 1) * blk_nodes, table_rows), :]
                            .rearrange("(r two) h -> r (two h)", two=2),
                            idx_sb[:, cell0 // 16: (cell0 + ncells) // 16],
                            ncells, ncells, 2 * H,
                            single_packet=False,
                        )
                        bufs.append(buf)
                    for t in range(t0, t1):
                        psum = ps_agg.tile([H, P], F32, space="PSUM",
                                           tag="pagg")
                        pl = group_pieces[gidx][t]
                        mi = 0
                        for (b, mcol, bufcol, p_) in pl:
                            buf = bufs[b]
                            mask = mk_mask(mcol)
                            nc.tensor.matmul(
                                out=psum[:],
                                lhsT=buf[:, bufcol, p_ * H:(p_ + 1) * H],
                                rhs=mask[:],
                                start=(mi == 0), stop=False,
                            )
                            mi += 1
                        mask = mk_mask(ncols_total + t)
                        nc.tensor.matmul(
                            out=psum[:],
                            lhsT=yshard[:, t, :],
                            rhs=mask[:],
                            start=(mi == 0), stop=True,
                        )
                        if ACT_EVAC:
                            nc.scalar.add(x_fm[:, t * P:(t + 1) * P],
                                          psum[:], b_sb[l][:])
                        else:
                            nc.vector.tensor_scalar(
                                x_fm[:, t * P:(t + 1) * P], psum[:],
                                b_sb[l][:], None, mybir.AluOpType.add,
                            )
                        if l + 1 < NL:
                            table_tile(l + 1, t)
                    if l + 1 < NL:
                        gi_write(l + 1, t0, t1)
                    else:
                        nc.sync.dma_start(
                            out=out_fm[:, t0 * P:t1 * P],
                            in_=x_fm[:, t0 * P:t1 * P],
                        )
                if l + 1 < NL:
                    distribute(l + 1)

    nc.compile()
    return nc


_CACHE = {}


def _prep_inputs(embeddings, edge_index, W1, b1, W2, b2, W3, b3):
    embeddings = np.ascontiguousarray(np.asarray(embeddings, dtype=np.float32))
    edge_index = np.asarray(edge_index)
    N = embeddings.shape[0]

    sched, core_tensors = _build_schedule(edge_index, N)
    shard_real, shard_p = sched["shard_real"], sched["shard_p"]

    key = (N, edge_index.shape[1], sched["total_cells"],
           tuple(sorted((k, v) for k, v in os.environ.items()
                        if k.startswith("GNN_"))))
    if key not in _CACHE:
        _CACHE[key] = _build_bass(sched)
    nc = _CACHE[key]

    iota = np.tile(np.arange(P, dtype=np.float16), (P, 1)).copy()
    ws = [np.asarray(W, np.float16) for W in (W1, W2, W3)]
    bs = [np.asarray(b, np.float32).reshape(H, 1) for b in (b1, b2, b3)]

    in_maps = []
    for c in range(NCORES):
        lo = c * shard_real
        hi = min(N, lo + shard_real)
        ct = core_tensors[c]
        emb_fm = np.zeros((H, shard_p), np.float16)
        emb_fm[:, ct["pos"]] = embeddings[lo:hi].T
        in_maps.append(dict(
            emb_fm=emb_fm, idx=ct["idx"], dst_rel=ct["dst_rel"],
            edinv=ct["edinv"], dinv_col=ct["dinv_col"], iota=iota,
            W1=ws[0], W2=ws[1], W3=ws[2], b1=bs[0], b2=bs[1], b3=bs[2],
        ))
    return nc, in_maps, sched, core_tensors


def kernel(embeddings, edge_index, W1, b1, W2, b2, W3, b3):
    N = np.asarray(embeddings).shape[0]
    nc, in_maps, sched, core_tensors = _prep_inputs(
        embeddings, edge_index, W1, b1, W2, b2, W3, b3)
    shard_real = sched["shard_real"]

    res = run_bass_kernel_spmd(nc, in_maps, core_ids=list(range(NCORES)))
    out = np.empty((N, H), np.float32)
    for c in range(NCORES):
        lo = c * shard_real
        hi = min(N, lo + shard_real)
        out[lo:hi] = res.results[c]["out_fm"].T[
            core_tensors[c]["pos"]].astype(np.float32)
    return out


def prepare(embeddings, edge_index, W1, b1, W2, b2, W3, b3):
    """Build (nc, in_maps) once for repeated benchmarking."""
    nc, in_maps, sched, _ct = _prep_inputs(
        embeddings, edge_index, W1, b1, W2, b2, W3, b3)
    return nc, in_maps, sched



# revision 2
# speedup vs baseline: 1.1588x; 1.1588x over previous
"""3-layer GCN (PyG GCNConv semantics) on 8 Trainium2 NeuronCores.

v3: the aggregation gathers 128B single-node rows (elem_size=64 fp16 at a
512B quad stride, 4 address classes = table-row % 4) instead of 256B node
pairs -- half the per-descriptor DMA cost.  One gather call per (tile-group,
class); matmul lhsT reads the full 128x64 buffer column (no parity half
select).  Otherwise identical to the proven pair-gather structure.

Math (per layer, matching the reference exactly):
    y = x @ W
    deg[d] = (# edges into d) + 1,  dinv = deg^-1/2
    out[d] = dinv[d] * ( sum_{e: dst=d} dinv[src_e] * y[src_e] + dinv[d]*y[d] ) + b
The norm dinv[src]*dinv[dst] is separable: fold dinv[src] into a per-node
table  ytab = dinv * (x @ W)  and dinv[dst] into the per-edge mask weight.
The self-loop term is just an extra edge (d, d).

Distribution: nodes (dsts) sharded across 8 cores; each core owns a
contiguous 12544-padded shard.  Each layer:
  1. table phase: core computes ytab for its shard (x_fm slice @ W, scaled),
     writes fp16 128B rows to DRAM, AllGather -> full table replica per core.
  2. aggregation: dma_gather with a PAIR view of the table (elem=128 fp16 =
     256B covering nodes 2k,2k+1; idx = src//2) streams per-edge rows into
     SBUF; per 128-cell column a selection mask (iota == dst_rel) * dinv[dst]
     routes cells to dst columns; PE matmuls (lhsT column half = src parity)
     accumulate a [64, 128] feature-major psum per dst tile.
Cells are laid out by (block, dst-tile, src-parity) segments padded to the
cross-core envelope (max over cores; node->slot assignment is balanced across
cores to keep the envelope near the mean); every 128-cell buffer column gets
per-segment mask columns so matmuls are full-range.  All cores run ONE shared
instruction schedule; per-core structure lives in data tensors (gather idxs,
dst_rel, edinv).
"""

import os
import numpy as np

import concourse.bacc as bacc
import concourse.mybir as mybir
import concourse.tile as tile
import concourse.ap_utils as ap_utils
from concourse.bass_utils import run_bass_kernel_spmd


def _narrow_gather(g, out_ap, in_ap, idxs_ap, num_idxs, elem_size, elem_step):
    """dma_gather with 128B elems at 512B stride (HW-verified); mirrors
    BassGpSimd.dma_gather minus the 256B-multiple elem assert."""
    assert idxs_ap.dtype == mybir.dt.int16
    assert in_ap.dtype == out_ap.dtype
    assert ap_utils.ap_is_contiguous(in_ap.ap[1:])
    assert ap_utils.ap_is_contiguous(out_ap.ap[1:])
    assert ap_utils.ap_is_contiguous(idxs_ap.ap[1:])
    assert in_ap.ap[-1][1] == elem_size and out_ap.ap[-1][1] == elem_size
    assert in_ap.ap[0][0] == elem_step
    stride_bytes = elem_step * mybir.dt.size(in_ap.dtype)
    assert stride_bytes % 256 == 0 and stride_bytes // 256 < 256
    _in_ap = g.lower_ap_dma(in_ap, for_custom_bir_dma=True)
    _idxs_ap = g.lower_ap(idxs_ap)
    _out_ap = g.lower_ap(out_ap)
    return g.add_instruction(
        mybir.InstDMAGatherAnt(
            name=g.bass.get_next_instruction_name(),
            ins=[*_in_ap, _idxs_ap, g.lower_val_access(g.to_reg(num_idxs))],
            outs=[_out_ap],
            transpose=False,
            num_idxs=num_idxs,
            elem_size=elem_size,
            stride_bytes_256=stride_bytes // 256,
            gen_mode=0,
            single_packet=False,
            queue_num=0,
        )
    )

P = 128
H = 64
NCORES = 8
GRAN = 1                  # envelope granularity (calls are 128-aligned anyway)
F32 = mybir.dt.float32
F16 = mybir.dt.float16


def _round_up(a, b):
    return (a + b - 1) // b * b


def _balanced_relabel(src, dst, N, shard_real, shard_p, degree):
    """Assign each node a position in its shard (tile = pos//128, parity =
    pos%2) so per-(core, src-block, dst-tile, src-parity) segment counts are
    balanced across cores.  Pass A packs dst nodes into tiles per core
    (balancing in-edge channel sums); pass B picks src parities (balancing
    the even/odd split of every segment globally).  Fallback: degree sort."""
    ntiles = shard_p // P
    nsh_blk = 4                                   # shards per src block
    nblocks = NCORES // nsh_blk
    core_of = dst // shard_real
    src_core = src // shard_real
    src_blk = src_core // nsh_blk

    if os.environ.get("GNN_BAL", "1") != "1":
        pos_list, newpos = [], np.empty(N, np.int64)
        for c in range(NCORES):
            lo, hi = c * shard_real, min(N, (c + 1) * shard_real)
            r = np.argsort(np.argsort(-degree[lo:hi], kind="stable"))
            pos_list.append(r)
            newpos[lo:hi] = r
        return pos_list, newpos

    rng = np.random.default_rng(0)
    # initial parity guess: alternate by degree rank within shard
    parity = np.empty(N, np.int8)
    for c in range(NCORES):
        lo, hi = c * shard_real, min(N, (c + 1) * shard_real)
        r = np.argsort(np.argsort(-degree[lo:hi], kind="stable"))
        parity[lo:hi] = (r % 2).astype(np.int8)

    # edges grouped by src for pass B
    src_order = np.argsort(src, kind="stable")
    src_sorted = src[src_order]
    src_starts = np.searchsorted(src_sorted, np.arange(N + 1))

    tile_of = np.empty(N, np.int64)
    for _ in range(2):
        # ---- pass A: per core, pack dst nodes into tiles (LPT greedy on
        # 4 channels = src_blk * 2 + src_parity, then swap refinement)
        ch = (src_blk * 2 + parity[src]).astype(np.int64)
        for c in range(NCORES):
            lo, hi = c * shard_real, min(N, (c + 1) * shard_real)
            m = core_of == c
            d_loc = dst[m] - lo
            nloc = hi - lo
            node_ch = np.zeros((nloc, 4), np.int64)
            np.add.at(node_ch, (d_loc, ch[m]), 1)
            tot = node_ch.sum(1)
            order = np.argsort(-tot, kind="stable")
            tsum = np.zeros((ntiles, 4), np.float64)
            cap = np.full(ntiles, P, np.int64)
            cap[-1] = P - (shard_p - nloc)         # pads live in last tile
            tgt = node_ch.sum(0).astype(np.float64) / ntiles
            tloc = np.empty(nloc, np.int64)
            for i in order:
                v = node_ch[i].astype(np.float64)
                score = ((tsum + v - tgt) ** 2).sum(1)
                score[cap <= 0] = np.inf
                t = int(np.argmin(score))
                tloc[i] = t
                tsum[t] += v
                cap[t] -= 1
            # swap refinement: move a node from the worst tile-channel to
            # the most complementary tile, swapping with its best partner
            nch = node_ch.astype(np.float64)
            for _r in range(4 * ntiles):
                dev = tsum - tgt
                t_hi = int(np.argmax(np.abs(dev).sum(1)))
                ids_hi = np.nonzero(tloc == t_hi)[0]
                t_lo = int(np.argmin((dev * dev[t_hi]).sum(1)))
                if t_lo == t_hi:
                    break
                ids_lo = np.nonzero(tloc == t_lo)[0]
                # best pair (i from hi, j from lo) minimizing new deviation
                di = nch[ids_hi]
                dj = nch[ids_lo]
                best = None
                cur_cost = (dev[t_hi] ** 2).sum() + (dev[t_lo] ** 2).sum()
                # evaluate a subsample for speed
                ii = ids_hi[np.argsort(-(di * dev[t_hi]).sum(1))[:16]]
                jj = ids_lo[np.argsort((dj * dev[t_hi]).sum(1))[:16]]
                for i_ in ii:
                    d_new_i = dev[t_hi] - nch[i_]
                    for j_ in jj:
                        dh = d_new_i + nch[j_]
                        dl = dev[t_lo] - nch[j_] + nch[i_]
                        cost = (dh ** 2).sum() + (dl ** 2).sum()
                        if best is None or cost < best[0]:
                            best = (cost, i_, j_)
                if best is None or best[0] >= cur_cost - 1e-9:
                    break
                _, i_, j_ = best
                tsum[t_hi] += nch[j_] - nch[i_]
                tsum[t_lo] += nch[i_] - nch[j_]
                tloc[i_], tloc[j_] = t_lo, t_hi
            tile_of[lo:hi] = tloc

        # ---- pass B: global greedy parity choice per src node (imbalance
        # tracked per (src-block, core, tile) segment; a src's block is fixed)
        imb = np.zeros((nblocks, NCORES * ntiles), np.float64)
        bins_dst = core_of * ntiles + tile_of[dst]    # seg bin of each edge
        ubins, umcount = {}, {}
        for u in rng.permutation(N):
            e0, e1 = src_starts[u], src_starts[u + 1]
            if e0 == e1:
                continue
            b, mcount = np.unique(bins_dst[src_order[e0:e1]],
                                  return_counts=True)
            ubins[u], umcount[u] = b, mcount
            bu = (u // shard_real) // nsh_blk
            cur = imb[bu, b]
            p_ = 0 if ((cur + mcount) ** 2).sum() <= ((cur - mcount) ** 2).sum() else 1
            parity[u] = p_
            imb[bu, b] += mcount if p_ == 0 else -mcount
        for _s in range(3):                        # refinement resweeps
            for u in ubins:
                b, mcount = ubins[u], umcount[u]
                bu = (u // shard_real) // nsh_blk
                sgn = 1 if parity[u] == 0 else -1
                cur = imb[bu, b] - sgn * mcount    # without u
                p_ = 0 if ((cur + mcount) ** 2).sum() <= ((cur - mcount) ** 2).sum() else 1
                if p_ != parity[u]:
                    parity[u] = p_
                    imb[bu, b] = cur + (mcount if p_ == 0 else -mcount)

    # ---- final slots: within each (core, tile), interleave by parity with
    # capacity 64/64 (overflow nodes take whatever slot class is left)
    pos_list, newpos = [], np.empty(N, np.int64)
    for c in range(NCORES):
        lo, hi = c * shard_real, min(N, (c + 1) * shard_real)
        nloc = hi - lo
        pos = np.empty(nloc, np.int64)
        for t in range(ntiles):
            ids = np.nonzero(tile_of[lo:hi] == t)[0]
            ev = ids[parity[lo + ids] == 0]
            od = ids[parity[lo + ids] == 1]
            if len(ev) > 64:
                od = np.concatenate([od, ev[64:]]); ev = ev[:64]
            if len(od) > 64:
                ev = np.concatenate([ev, od[64:]]); od = od[:64]
            pos[ev] = t * P + 2 * np.arange(len(ev))
            pos[od] = t * P + 2 * np.arange(len(od)) + 1
        pos_list.append(pos)
        newpos[lo:hi] = pos
    return pos_list, newpos


# ----------------------------------------------------------------------------
# host-side schedule + per-core tensors
# ----------------------------------------------------------------------------
def _build_schedule(edge_index, N):
    src = np.asarray(edge_index[0], dtype=np.int64)
    dst = np.asarray(edge_index[1], dtype=np.int64)

    shard_real = (N + NCORES - 1) // NCORES           # 12500
    shard_p = _round_up(shard_real, P)                # 12544
    ntiles = shard_p // P                             # 98
    blk_nodes = 4 * shard_p                           # 50176 (25088 pairs < 2^15)
    nblocks = (NCORES * shard_p + blk_nodes - 1) // blk_nodes  # 2

    degree = np.bincount(dst, minlength=N).astype(np.int64)
    deg = degree.astype(np.float64) + 1.0
    dinv = (1.0 / np.sqrt(deg)).astype(np.float32)

    # Relabeling per shard: pick each node's (tile, even/odd slot) to balance
    # the per-(core, block, tile, parity) segment counts across cores, so the
    # cross-core envelope (max over cores) stays near the mean.
    pos_list, newpos = _balanced_relabel(src, dst, N, shard_real, shard_p, degree)

    core_of = dst // shard_real
    src_gid = (src // shard_real) * shard_p + newpos[src]
    ldst = newpos[dst]

    blk = (src_gid // 2) % 2          # par4 high bit
    pair_loc = src_gid // 4           # quad index (int16-safe, < 25088)
    par = src_gid % 2                 # par4 low bit
    tile_id = ldst // P
    seg = (blk * ntiles + tile_id) * 2 + par          # (b, t, par) segment
    nseg = nblocks * ntiles * 2

    counts = np.zeros((NCORES, nseg), np.int64)
    per_core = []
    for c in range(NCORES):
        m = core_of == c
        s_c = seg[m]
        counts[c] = np.bincount(s_c, minlength=nseg)
        per_core.append((s_c, pair_loc[m], ldst[m], dst[m]))

    smax = counts.max(0)
    S_seg = _round_up(smax, GRAN)                     # padded cells per segment

    def seg_id(b, t, p_):
        return (b * ntiles + t) * 2 + p_

    def seg_sum(b, ta, tb_):
        return int(S_seg[seg_id(b, ta, 0):seg_id(b, tb_ - 1, 1) + 1].sum())

    # ---- tile groups: per (group, block) one gather call of <= MCB columns;
    # boundaries chosen by DP to minimize the 128-alignment padding of calls
    MCB = int(os.environ.get("GNN_MCB", "36"))
    pref = np.zeros((nblocks, ntiles + 1), np.int64)
    for b in range(nblocks):
        for t in range(ntiles):
            pref[b, t + 1] = pref[b, t] + S_seg[seg_id(b, t, 0)] + \
                S_seg[seg_id(b, t, 1)]
    LAM = 32                                      # small bias to fewer calls
    best = np.full(ntiles + 1, 1 << 60, np.int64)
    prev = np.full(ntiles + 1, -1, np.int64)
    best[0] = 0
    for t1 in range(1, ntiles + 1):
        for t0 in range(t1 - 1, -1, -1):
            ss = pref[:, t1] - pref[:, t0]
            if (-(-ss // P)).max() > MCB:
                break
            pad = int(((-ss) % P).sum()) + LAM
            if best[t0] + pad < best[t1]:
                best[t1] = best[t0] + pad
                prev[t1] = t0
    tile_groups = []
    t1 = ntiles
    while t1 > 0:
        t0 = int(prev[t1])
        tile_groups.append((t0, t1))
        t1 = t0
    tile_groups.reverse()

    # ---- cell layout: calls laid out in (group, block) order, each call
    # 128-aligned so the gather writes every row any matmul reads.
    base = np.zeros(nseg, np.int64)
    calls = []          # per group: list over (b*2+p) classes of (cell0, ncells)
    cur = 0
    for (t0, t1) in tile_groups:
        cinfo = []
        for b in range(nblocks):
            for p_ in range(2):
                cell0 = cur
                for t in range(t0, t1):
                    s = seg_id(b, t, p_)
                    base[s] = cur
                    cur += int(S_seg[s])
                cur = _round_up(cur, P)
                cinfo.append((cell0, cur - cell0))
        calls.append(cinfo)
    total_cells = cur

    # ---- static piece table: one full-range matmul per (segment, buf column)
    # with a dedicated mask column (rows outside the segment span disabled).
    group_pieces = []   # per group: {t: [(b, mask_col, buf_col, parity)]}
    mc_base = np.zeros(nseg, np.int64)   # first mask col of each segment
    mc = 0
    for gidx, (t0, t1) in enumerate(tile_groups):
        cinfo = calls[gidx]
        pieces = {}
        for t in range(t0, t1):
            pl = []
            for b in range(nblocks):
                for p_ in range(2):
                    c0 = cinfo[b * 2 + p_][0]
                    s = seg_id(b, t, p_)
                    a0 = int(base[s]) - c0
                    a1 = a0 + int(S_seg[s])
                    mc_base[s] = mc
                    for col in range(a0 // P, -(-a1 // P)):
                        pl.append((b, mc, col, p_))
                        mc += 1
            pieces[t] = pl
        group_pieces.append(pieces)
    ncols_total = mc

    # ---- per-core cell tensors
    core_tensors = []
    # per-cell mapping: global pos -> call-local (partition, buf col, mask col)
    call_cell0 = np.sort(np.array(
        [cinfo[k][0] for cinfo in calls for k in range(2 * nblocks)], np.int64))

    def cell_to_rowcol(pos, s_s):
        ci = np.searchsorted(call_cell0, pos, side="right") - 1
        local = pos - call_cell0[ci]
        col = local // P
        a0col = (base[s_s] - call_cell0[ci]) // P
        return (local % P).astype(np.int64), mc_base[s_s] + col - a0col

    for c in range(NCORES):
        s_c, pl_c, ld_c, d_c = per_core[c]
        order = np.argsort(s_c, kind="stable")
        s_s = s_c[order]
        cnt = counts[c]
        starts = np.zeros(nseg, np.int64)
        np.cumsum(cnt[:-1], out=starts[1:])
        rank = np.arange(s_s.shape[0], dtype=np.int64) - np.repeat(starts, cnt)
        pos = base[s_s] + rank
        prow, mcol = cell_to_rowcol(pos, s_s)

        idx_cells = np.zeros(total_cells, np.int16)          # pad -> pair 0
        dstrel = np.full((P, ncols_total + ntiles), -1.0, np.float32)
        edinv = np.zeros((P, ncols_total + ntiles), np.float32)
        idx_cells[pos] = pl_c[order].astype(np.int16)
        dstrel[prow, mcol] = (ld_c[order] % P).astype(np.float32)
        edinv[prow, mcol] = dinv[d_c[order]].astype(np.float32)

        idx16 = idx_cells.reshape(-1, 16).T                  # [16, cells/16]
        idx_w = np.tile(idx16, (8, 1)).copy()                # [128, cells/16]

        lo = c * shard_real
        hi = min(N, lo + shard_real)
        dvec = np.ones(shard_p, np.float32)
        dvec[pos_list[c]] = dinv[lo:hi]
        dinv_col = dvec.reshape(ntiles, P).T.astype(np.float32).copy()

        # self columns (one per tile, after gather cols)
        valid = np.zeros(shard_p, bool)
        valid[pos_list[c]] = True
        sd = np.full(shard_p, -1.0, np.float32)
        se = np.zeros(shard_p, np.float32)
        sd[valid] = (np.arange(shard_p) % P).astype(np.float32)[valid]
        se[valid] = dvec[valid]
        dstrel[:, ncols_total:] = sd.reshape(ntiles, P).T
        edinv[:, ncols_total:] = se.reshape(ntiles, P).T

        core_tensors.append(
            dict(idx=idx_w, dst_rel=np.ascontiguousarray(dstrel),
                 edinv=np.ascontiguousarray(edinv), dinv_col=dinv_col,
                 pos=pos_list[c])
        )

    sched = dict(
        shard_real=shard_real,
        shard_p=shard_p,
        blk_nodes=blk_nodes,
        nblocks=nblocks,
        ntiles=ntiles,
        tile_groups=tile_groups,
        calls=calls,
        group_pieces=group_pieces,
        ncols_total=ncols_total,
        total_cells=total_cells,
    )
    return sched, core_tensors


# ----------------------------------------------------------------------------
# bass builder
# ----------------------------------------------------------------------------
def _build_bass(sched):
    shard_p = sched["shard_p"]
    blk_nodes = sched["blk_nodes"]
    nblocks = sched["nblocks"]
    ntiles = sched["ntiles"]
    tile_groups = sched["tile_groups"]
    calls = sched["calls"]
    group_pieces = sched["group_pieces"]
    ncols_total = sched["ncols_total"]
    total_cells = sched["total_cells"]
    table_rows = NCORES * shard_p
    blk_pairs = blk_nodes // 2

    TSIM = bool(int(os.environ.get("GNN_TSIM", "0")))
    nc = bacc.Bacc("TRN2", target_bir_lowering=False,
                   num_devices=1 if TSIM else NCORES,
                   dynamic_dma_scratch_size=int(os.environ.get("GNN_SCRATCH", "65536")))

    emb_in = nc.dram_tensor("emb_fm", [H, shard_p], F16, kind="ExternalInput")
    idx_in = nc.dram_tensor("idx", [P, total_cells // 16], mybir.dt.int16,
                            kind="ExternalInput")
    dst_rel_in = nc.dram_tensor("dst_rel", [P, ncols_total + ntiles], F32,
                                kind="ExternalInput")
    edinv_in = nc.dram_tensor("edinv", [P, ncols_total + ntiles], F32,
                              kind="ExternalInput")
    dinv_col_in = nc.dram_tensor("dinv_col", [P, ntiles], F32, kind="ExternalInput")
    iota_in = nc.dram_tensor("iota", [P, P], F16, kind="ExternalInput")
    w_ins = [nc.dram_tensor(f"W{l+1}", [H, H], F16, kind="ExternalInput")
             for l in range(3)]
    b_ins = [nc.dram_tensor(f"b{l+1}", [H, 1], F32, kind="ExternalInput")
             for l in range(3)]
    out_fm = nc.dram_tensor("out_fm", [H, shard_p], F16, kind="ExternalOutput")

    NL = int(os.environ.get("GNN_NLAYERS", "3"))
    ACT_EVAC = os.environ.get("GNN_ACT", "0") == "1"
    MGB = int(os.environ.get("GNN_MGB", "3"))

    with tile.TileContext(nc) as tc:
        with (
            tc.tile_pool(name="persist", bufs=1) as persist,
            tc.tile_pool(name="msgs", bufs=MGB) as msgs_pool,
            tc.tile_pool(name="masks", bufs=int(os.environ.get("GNN_MKB", "24"))) as mask_pool,
            tc.tile_pool(name="ps_agg", bufs=int(os.environ.get("GNN_PSB", "6")), space="PSUM") as ps_agg,
            tc.tile_pool(name="ps_tb", bufs=2, space="PSUM") as ps_tb,
            tc.tile_pool(name="dram", bufs=1, space="DRAM") as dram,
        ):
            # ---- persistent SBUF ----
            x_fm = persist.tile([H, shard_p], F16)
            yshard = persist.tile([P, ntiles, H], F16)
            idx_sb = persist.tile([P, total_cells // 16], mybir.dt.int16)
            dst_rel = persist.tile([P, ncols_total + ntiles], F32)
            edinv = persist.tile([P, ncols_total + ntiles], F32)
            dinv_col = persist.tile([P, ntiles], F32)
            iota_sb = persist.tile([P, P], F16)
            w_sb = [persist.tile([H, H], F16, name=f"w{l}") for l in range(3)]
            b_sb = [persist.tile([H, 1], F32, name=f"b{l}") for l in range(3)]

            nc.sync.dma_start(out=x_fm[:], in_=emb_in[:])
            nc.sync.dma_start(out=idx_sb[:], in_=idx_in[:])
            nc.sync.dma_start(out=dst_rel[:], in_=dst_rel_in[:])
            nc.sync.dma_start(out=edinv[:], in_=edinv_in[:])
            nc.sync.dma_start(out=dinv_col[:], in_=dinv_col_in[:])
            nc.sync.dma_start(out=iota_sb[:], in_=iota_in[:])
            for l in range(3):
                nc.sync.dma_start(out=w_sb[l][:], in_=w_ins[l][:])
                nc.sync.dma_start(out=b_sb[l][:], in_=b_ins[l][:])

            ag_in = [dram.tile([shard_p, H], F16, name=f"agin{i}") for i in range(3)]
            tables = [dram.tile([table_rows, H], F16, addr_space="Shared",
                                name=f"table{i}") for i in range(3)]

            def mk_mask(col):
                mask = mask_pool.tile([P, P], F16, tag="mask")
                nc.vector.tensor_scalar(
                    mask[:], iota_sb[:],
                    dst_rel[:, col:col + 1],
                    edinv[:, col:col + 1],
                    mybir.AluOpType.is_equal,
                    mybir.AluOpType.mult,
                )
                return mask

            def table_tile(l, t):
                pt = ps_tb.tile([P, H], F32, space="PSUM", tag="pt")
                nc.tensor.matmul(
                    out=pt[:],
                    lhsT=x_fm[:, t * P:(t + 1) * P],
                    rhs=w_sb[l][:],
                    start=True, stop=True,
                )
                if ACT_EVAC:
                    nc.scalar.mul(yshard[:, t, :], pt[:], dinv_col[:, t:t + 1])
                else:
                    nc.vector.tensor_scalar_mul(
                        yshard[:, t, :], pt[:], dinv_col[:, t:t + 1])

            def gi_write(l, t0, t1):
                nc.sync.dma_start(
                    out=ag_in[l][t0 * P:t1 * P, :].rearrange(
                        "(a p) h -> p a h", p=P),
                    in_=yshard[:, t0:t1, :],
                )

            def distribute(l):
                if TSIM:
                    nc.sync.dma_start(out=tables[l][0:shard_p, :],
                                      in_=ag_in[l][:])
                else:
                    nc.gpsimd.collective_compute(
                        "AllGather",
                        mybir.AluOpType.bypass,
                        replica_groups=[list(range(NCORES))],
                        ins=[ag_in[l][:].opt()],
                        outs=[tables[l][: NCORES * shard_p, :].opt()],
                    )

            # ---- prologue: layer-0 table from the embeddings ----
            for t in range(ntiles):
                table_tile(0, t)
            for t0 in range(0, ntiles, 14):
                gi_write(0, t0, min(ntiles, t0 + 14))
            distribute(0)

            # emit groups largest-first so the smallest group drains the
            # pipeline at each layer boundary (shortest AllGather stall)
            emit_order = sorted(
                range(len(tile_groups)),
                key=lambda g: -max(c[1] for c in calls[g]))
            for l in range(NL):
                tb = tables[l]
                # ---- aggregation (tile groups; pair-view gather); the next
                # layer's table phase is interleaved per tile so only the
                # AllGather remains at the layer boundary ----
                tb4 = tb[:].rearrange("(r four) h -> r (four h)", four=4)
                for gidx in emit_order:
                    t0, t1 = tile_groups[gidx]
                    cinfo = calls[gidx]
                    bufs = []
                    for k in range(2 * nblocks):
                        cell0, ncells = cinfo[k]
                        if ncells == 0:
                            bufs.append(None)
                            continue
                        ncols = -(-ncells // P)
                        buf = msgs_pool.tile([P, ncols, H], F16,
                                             tag=f"msgs{k}", bufs=MGB)
                        _narrow_gather(
                            nc.gpsimd, buf[:],
                            tb4[:, k * H:(k + 1) * H],
                            idx_sb[:, cell0 // 16: (cell0 + ncells) // 16],
                            ncells, H, 4 * H,
                        )
                        bufs.append(buf)
                    for t in range(t0, t1):
                        psum = ps_agg.tile([H, P], F32, space="PSUM",
                                           tag="pagg")
                        pl = group_pieces[gidx][t]
                        mi = 0
                        for (b, mcol, bufcol, p_) in pl:
                            buf = bufs[b * 2 + p_]
                            mask = mk_mask(mcol)
                            nc.tensor.matmul(
                                out=psum[:],
                                lhsT=buf[:, bufcol, :],
                                rhs=mask[:],
                                start=(mi == 0), stop=False,
                            )
                            mi += 1
                        mask = mk_mask(ncols_total + t)
                        nc.tensor.matmul(
                            out=psum[:],
                            lhsT=yshard[:, t, :],
                            rhs=mask[:],
                            start=(mi == 0), stop=True,
                        )
                        if ACT_EVAC:
                            nc.scalar.add(x_fm[:, t * P:(t + 1) * P],
                                          psum[:], b_sb[l][:])
                        else:
                            nc.vector.tensor_scalar(
                                x_fm[:, t * P:(t + 1) * P], psum[:],
                                b_sb[l][:], None, mybir.AluOpType.add,
                            )
                        if l + 1 < NL:
                            table_tile(l + 1, t)
                    if l + 1 < NL:
                        gi_write(l + 1, t0, t1)
                    else:
                        nc.sync.dma_start(
                            out=out_fm[:, t0 * P:t1 * P],
                            in_=x_fm[:, t0 * P:t1 * P],
                        )
                if l + 1 < NL:
                    distribute(l + 1)

    nc.compile()
    return nc


_CACHE = {}


def _prep_inputs(embeddings, edge_index, W1, b1, W2, b2, W3, b3):
    embeddings = np.ascontiguousarray(np.asarray(embeddings, dtype=np.float32))
    edge_index = np.asarray(edge_index)
    N = embeddings.shape[0]

    sched, core_tensors = _build_schedule(edge_index, N)
    shard_real, shard_p = sched["shard_real"], sched["shard_p"]

    key = (N, edge_index.shape[1], sched["total_cells"],
           tuple(sorted((k, v) for k, v in os.environ.items()
                        if k.startswith("GNN_"))))
    if key not in _CACHE:
        _CACHE[key] = _build_bass(sched)
    nc = _CACHE[key]

    iota = np.tile(np.arange(P, dtype=np.float16), (P, 1)).copy()
    ws = [np.asarray(W, np.float16) for W in (W1, W2, W3)]
    bs = [np.asarray(b, np.float32).reshape(H, 1) for b in (b1, b2, b3)]

    in_maps = []
    for c in range(NCORES):
        lo = c * shard_real
        hi = min(N, lo + shard_real)
        ct = core_tensors[c]
        emb_fm = np.zeros((H, shard_p), np.float16)
        emb_fm[:, ct["pos"]] = embeddings[lo:hi].T
        in_maps.append(dict(
            emb_fm=emb_fm, idx=ct["idx"], dst_rel=ct["dst_rel"],
            edinv=ct["edinv"], dinv_col=ct["dinv_col"], iota=iota,
            W1=ws[0], W2=ws[1], W3=ws[2], b1=bs[0], b2=bs[1], b3=bs[2],
        ))
    return nc, in_maps, sched, core_tensors


def kernel(embeddings, edge_index, W1, b1, W2, b2, W3, b3):
    N = np.asarray(embeddings).shape[0]
    nc, in_maps, sched, core_tensors = _prep_inputs(
        embeddings, edge_index, W1, b1, W2, b2, W3, b3)
    shard_real = sched["shard_real"]

    res = run_bass_kernel_spmd(nc, in_maps, core_ids=list(range(NCORES)))
    out = np.empty((N, H), np.float32)
    for c in range(NCORES):
        lo = c * shard_real
        hi = min(N, lo + shard_real)
        out[lo:hi] = res.results[c]["out_fm"].T[
            core_tensors[c]["pos"]].astype(np.float32)
    return out


def prepare(embeddings, edge_index, W1, b1, W2, b2, W3, b3):
    """Build (nc, in_maps) once for repeated benchmarking."""
    nc, in_maps, sched, _ct = _prep_inputs(
        embeddings, edge_index, W1, b1, W2, b2, W3, b3)
    return nc, in_maps, sched



# revision 4
# speedup vs baseline: 1.1608x; 1.0017x over previous
"""3-layer GCN (PyG GCNConv semantics) on 8 Trainium2 NeuronCores.

v3: the aggregation gathers 128B single-node rows (elem_size=64 fp16 at a
512B quad stride, 4 address classes = table-row % 4) instead of 256B node
pairs -- half the per-descriptor DMA cost.  One gather call per (tile-group,
class); matmul lhsT reads the full 128x64 buffer column (no parity half
select).  Otherwise identical to the proven pair-gather structure.

Math (per layer, matching the reference exactly):
    y = x @ W
    deg[d] = (# edges into d) + 1,  dinv = deg^-1/2
    out[d] = dinv[d] * ( sum_{e: dst=d} dinv[src_e] * y[src_e] + dinv[d]*y[d] ) + b
The norm dinv[src]*dinv[dst] is separable: fold dinv[src] into a per-node
table  ytab = dinv * (x @ W)  and dinv[dst] into the per-edge mask weight.
The self-loop term is just an extra edge (d, d).

Distribution: nodes (dsts) sharded across 8 cores; each core owns a
contiguous 12544-padded shard.  Each layer:
  1. table phase: core computes ytab for its shard (x_fm slice @ W, scaled),
     writes fp16 128B rows to DRAM, AllGather -> full table replica per core.
  2. aggregation: dma_gather with a PAIR view of the table (elem=128 fp16 =
     256B covering nodes 2k,2k+1; idx = src//2) streams per-edge rows into
     SBUF; per 128-cell column a selection mask (iota == dst_rel) * dinv[dst]
     routes cells to dst columns; PE matmuls (lhsT column half = src parity)
     accumulate a [64, 128] feature-major psum per dst tile.
Cells are laid out by (block, dst-tile, src-parity) segments padded to the
cross-core envelope (max over cores; node->slot assignment is balanced across
cores to keep the envelope near the mean); every 128-cell buffer column gets
per-segment mask columns so matmuls are full-range.  All cores run ONE shared
instruction schedule; per-core structure lives in data tensors (gather idxs,
dst_rel, edinv).
"""

import os
import numpy as np

import concourse.bacc as bacc
import concourse.mybir as mybir
import concourse.tile as tile
import concourse.ap_utils as ap_utils
from concourse.bass_utils import run_bass_kernel_spmd


def _narrow_gather(g, out_ap, in_ap, idxs_ap, num_idxs, elem_size, elem_step):
    """dma_gather with 128B elems at 512B stride (HW-verified); mirrors
    BassGpSimd.dma_gather minus the 256B-multiple elem assert."""
    assert idxs_ap.dtype == mybir.dt.int16
    assert in_ap.dtype == out_ap.dtype
    assert ap_utils.ap_is_contiguous(in_ap.ap[1:])
    assert ap_utils.ap_is_contiguous(out_ap.ap[1:])
    assert ap_utils.ap_is_contiguous(idxs_ap.ap[1:])
    assert in_ap.ap[-1][1] == elem_size and out_ap.ap[-1][1] == elem_size
    assert in_ap.ap[0][0] == elem_step
    stride_bytes = elem_step * mybir.dt.size(in_ap.dtype)
    assert stride_bytes % 256 == 0 and stride_bytes // 256 < 256
    _in_ap = g.lower_ap_dma(in_ap, for_custom_bir_dma=True)
    _idxs_ap = g.lower_ap(idxs_ap)
    _out_ap = g.lower_ap(out_ap)
    return g.add_instruction(
        mybir.InstDMAGatherAnt(
            name=g.bass.get_next_instruction_name(),
            ins=[*_in_ap, _idxs_ap, g.lower_val_access(g.to_reg(num_idxs))],
            outs=[_out_ap],
            transpose=False,
            num_idxs=num_idxs,
            elem_size=elem_size,
            stride_bytes_256=stride_bytes // 256,
            gen_mode=0,
            single_packet=False,
            queue_num=0,
        )
    )

P = 128
H = 64
NCORES = 8
GRAN = 1                  # envelope granularity (calls are 128-aligned anyway)
F32 = mybir.dt.float32
F16 = mybir.dt.float16


def _round_up(a, b):
    return (a + b - 1) // b * b


def _balanced_relabel(src, dst, N, shard_real, shard_p, degree):
    """Assign each node a position in its shard (tile = pos//128, parity =
    pos%2) so per-(core, src-block, dst-tile, src-parity) segment counts are
    balanced across cores.  Pass A packs dst nodes into tiles per core
    (balancing in-edge channel sums); pass B picks src parities (balancing
    the even/odd split of every segment globally).  Fallback: degree sort."""
    ntiles = shard_p // P
    nsh_blk = 4                                   # shards per src block
    nblocks = NCORES // nsh_blk
    core_of = dst // shard_real
    src_core = src // shard_real
    src_blk = src_core // nsh_blk

    if os.environ.get("GNN_BAL", "1") != "1":
        pos_list, newpos = [], np.empty(N, np.int64)
        for c in range(NCORES):
            lo, hi = c * shard_real, min(N, (c + 1) * shard_real)
            r = np.argsort(np.argsort(-degree[lo:hi], kind="stable"))
            pos_list.append(r)
            newpos[lo:hi] = r
        return pos_list, newpos

    rng = np.random.default_rng(0)
    # initial parity guess: alternate by degree rank within shard
    parity = np.empty(N, np.int8)
    for c in range(NCORES):
        lo, hi = c * shard_real, min(N, (c + 1) * shard_real)
        r = np.argsort(np.argsort(-degree[lo:hi], kind="stable"))
        parity[lo:hi] = (r % 2).astype(np.int8)

    # edges grouped by src for pass B
    src_order = np.argsort(src, kind="stable")
    src_sorted = src[src_order]
    src_starts = np.searchsorted(src_sorted, np.arange(N + 1))

    tile_of = np.empty(N, np.int64)
    for _ in range(2):
        # ---- pass A: per core, pack dst nodes into tiles (LPT greedy on
        # 4 channels = src_blk * 2 + src_parity, then swap refinement)
        ch = (src_blk * 2 + parity[src]).astype(np.int64)
        for c in range(NCORES):
            lo, hi = c * shard_real, min(N, (c + 1) * shard_real)
            m = core_of == c
            d_loc = dst[m] - lo
            nloc = hi - lo
            node_ch = np.zeros((nloc, 4), np.int64)
            np.add.at(node_ch, (d_loc, ch[m]), 1)
            tot = node_ch.sum(1)
            order = np.argsort(-tot, kind="stable")
            tsum = np.zeros((ntiles, 4), np.float64)
            cap = np.full(ntiles, P, np.int64)
            cap[-1] = P - (shard_p - nloc)         # pads live in last tile
            tgt = node_ch.sum(0).astype(np.float64) / ntiles
            tloc = np.empty(nloc, np.int64)
            for i in order:
                v = node_ch[i].astype(np.float64)
                score = ((tsum + v - tgt) ** 2).sum(1)
                score[cap <= 0] = np.inf
                t = int(np.argmin(score))
                tloc[i] = t
                tsum[t] += v
                cap[t] -= 1
            # swap refinement: move a node from the worst tile-channel to
            # the most complementary tile, swapping with its best partner
            nch = node_ch.astype(np.float64)
            for _r in range(4 * ntiles):
                dev = tsum - tgt
                t_hi = int(np.argmax(np.abs(dev).sum(1)))
                ids_hi = np.nonzero(tloc == t_hi)[0]
                t_lo = int(np.argmin((dev * dev[t_hi]).sum(1)))
                if t_lo == t_hi:
                    break
                ids_lo = np.nonzero(tloc == t_lo)[0]
                # best pair (i from hi, j from lo) minimizing new deviation
                di = nch[ids_hi]
                dj = nch[ids_lo]
                best = None
                cur_cost = (dev[t_hi] ** 2).sum() + (dev[t_lo] ** 2).sum()
                # evaluate a subsample for speed
                ii = ids_hi[np.argsort(-(di * dev[t_hi]).sum(1))[:16]]
                jj = ids_lo[np.argsort((dj * dev[t_hi]).sum(1))[:16]]
                for i_ in ii:
                    d_new_i = dev[t_hi] - nch[i_]
                    for j_ in jj:
                        dh = d_new_i + nch[j_]
                        dl = dev[t_lo] - nch[j_] + nch[i_]
                        cost = (dh ** 2).sum() + (dl ** 2).sum()
                        if best is None or cost < best[0]:
                            best = (cost, i_, j_)
                if best is None or best[0] >= cur_cost - 1e-9:
                    break
                _, i_, j_ = best
                tsum[t_hi] += nch[j_] - nch[i_]
                tsum[t_lo] += nch[i_] - nch[j_]
                tloc[i_], tloc[j_] = t_lo, t_hi
            tile_of[lo:hi] = tloc

        # ---- pass B: global greedy parity choice per src node (imbalance
        # tracked per (src-block, core, tile) segment; a src's block is fixed)
        imb = np.zeros((nblocks, NCORES * ntiles), np.float64)
        bins_dst = core_of * ntiles + tile_of[dst]    # seg bin of each edge
        ubins, umcount = {}, {}
        for u in rng.permutation(N):
            e0, e1 = src_starts[u], src_starts[u + 1]
            if e0 == e1:
                continue
            b, mcount = np.unique(bins_dst[src_order[e0:e1]],
                                  return_counts=True)
            ubins[u], umcount[u] = b, mcount
            bu = (u // shard_real) // nsh_blk
            cur = imb[bu, b]
            p_ = 0 if ((cur + mcount) ** 2).sum() <= ((cur - mcount) ** 2).sum() else 1
            parity[u] = p_
            imb[bu, b] += mcount if p_ == 0 else -mcount
        for _s in range(3):                        # refinement resweeps
            for u in ubins:
                b, mcount = ubins[u], umcount[u]
                bu = (u // shard_real) // nsh_blk
                sgn = 1 if parity[u] == 0 else -1
                cur = imb[bu, b] - sgn * mcount    # without u
                p_ = 0 if ((cur + mcount) ** 2).sum() <= ((cur - mcount) ** 2).sum() else 1
                if p_ != parity[u]:
                    parity[u] = p_
                    imb[bu, b] = cur + (mcount if p_ == 0 else -mcount)

    # ---- final slots: within each (core, tile), interleave by parity with
    # capacity 64/64 (overflow nodes take whatever slot class is left)
    pos_list, newpos = [], np.empty(N, np.int64)
    for c in range(NCORES):
        lo, hi = c * shard_real, min(N, (c + 1) * shard_real)
        nloc = hi - lo
        pos = np.empty(nloc, np.int64)
        for t in range(ntiles):
            ids = np.nonzero(tile_of[lo:hi] == t)[0]
            ev = ids[parity[lo + ids] == 0]
            od = ids[parity[lo + ids] == 1]
            if len(ev) > 64:
                od = np.concatenate([od, ev[64:]]); ev = ev[:64]
            if len(od) > 64:
                ev = np.concatenate([ev, od[64:]]); od = od[:64]
            pos[ev] = t * P + 2 * np.arange(len(ev))
            pos[od] = t * P + 2 * np.arange(len(od)) + 1
        pos_list.append(pos)
        newpos[lo:hi] = pos
    return pos_list, newpos


# ----------------------------------------------------------------------------
# host-side schedule + per-core tensors
# ----------------------------------------------------------------------------
def _build_schedule(edge_index, N):
    src = np.asarray(edge_index[0], dtype=np.int64)
    dst = np.asarray(edge_index[1], dtype=np.int64)

    shard_real = (N + NCORES - 1) // NCORES           # 12500
    shard_p = _round_up(shard_real, P)                # 12544
    ntiles = shard_p // P                             # 98
    blk_nodes = 4 * shard_p                           # 50176 (25088 pairs < 2^15)
    nblocks = (NCORES * shard_p + blk_nodes - 1) // blk_nodes  # 2

    degree = np.bincount(dst, minlength=N).astype(np.int64)
    deg = degree.astype(np.float64) + 1.0
    dinv = (1.0 / np.sqrt(deg)).astype(np.float32)

    # Relabeling per shard: pick each node's (tile, even/odd slot) to balance
    # the per-(core, block, tile, parity) segment counts across cores, so the
    # cross-core envelope (max over cores) stays near the mean.
    pos_list, newpos = _balanced_relabel(src, dst, N, shard_real, shard_p, degree)

    core_of = dst // shard_real
    src_gid = (src // shard_real) * shard_p + newpos[src]
    ldst = newpos[dst]

    blk = (src_gid // 2) % 2          # par4 high bit
    pair_loc = src_gid // 4           # quad index (int16-safe, < 25088)
    par = src_gid % 2                 # par4 low bit
    tile_id = ldst // P
    seg = (blk * ntiles + tile_id) * 2 + par          # (b, t, par) segment
    nseg = nblocks * ntiles * 2

    counts = np.zeros((NCORES, nseg), np.int64)
    per_core = []
    for c in range(NCORES):
        m = core_of == c
        s_c = seg[m]
        counts[c] = np.bincount(s_c, minlength=nseg)
        per_core.append((s_c, pair_loc[m], ldst[m], dst[m]))

    smax = counts.max(0)
    S_seg = _round_up(smax, GRAN)                     # padded cells per segment

    def seg_id(b, t, p_):
        return (b * ntiles + t) * 2 + p_

    def seg_sum(b, ta, tb_):
        return int(S_seg[seg_id(b, ta, 0):seg_id(b, tb_ - 1, 1) + 1].sum())

    # ---- tile groups: per (group, block) one gather call of <= MCB columns;
    # boundaries chosen by DP to minimize the 128-alignment padding of calls
    MCB = int(os.environ.get("GNN_MCB", "36"))
    pref = np.zeros((nblocks, ntiles + 1), np.int64)
    for b in range(nblocks):
        for t in range(ntiles):
            pref[b, t + 1] = pref[b, t] + S_seg[seg_id(b, t, 0)] + \
                S_seg[seg_id(b, t, 1)]
    LAM = 32                                      # small bias to fewer calls
    best = np.full(ntiles + 1, 1 << 60, np.int64)
    prev = np.full(ntiles + 1, -1, np.int64)
    best[0] = 0
    for t1 in range(1, ntiles + 1):
        for t0 in range(t1 - 1, -1, -1):
            ss = pref[:, t1] - pref[:, t0]
            if (-(-ss // P)).max() > MCB:
                break
            pad = int(((-ss) % P).sum()) + LAM
            if best[t0] + pad < best[t1]:
                best[t1] = best[t0] + pad
                prev[t1] = t0
    tile_groups = []
    t1 = ntiles
    while t1 > 0:
        t0 = int(prev[t1])
        tile_groups.append((t0, t1))
        t1 = t0
    tile_groups.reverse()

    # ---- cell layout: calls laid out in (group, block) order, each call
    # 128-aligned so the gather writes every row any matmul reads.
    base = np.zeros(nseg, np.int64)
    calls = []          # per group: list over (b*2+p) classes of (cell0, ncells)
    cur = 0
    for (t0, t1) in tile_groups:
        cinfo = []
        for b in range(nblocks):
            for p_ in range(2):
                cell0 = cur
                for t in range(t0, t1):
                    s = seg_id(b, t, p_)
                    base[s] = cur
                    cur += int(S_seg[s])
                cur = _round_up(cur, P)
                cinfo.append((cell0, cur - cell0))
        calls.append(cinfo)
    total_cells = cur

    # ---- static piece table: one full-range matmul per (segment, buf column)
    # with a dedicated mask column (rows outside the segment span disabled).
    group_pieces = []   # per group: {t: [(b, mask_col, buf_col, parity)]}
    mc_base = np.zeros(nseg, np.int64)   # first mask col of each segment
    mc = 0
    for gidx, (t0, t1) in enumerate(tile_groups):
        cinfo = calls[gidx]
        pieces = {}
        for t in range(t0, t1):
            pl = []
            for b in range(nblocks):
                for p_ in range(2):
                    c0 = cinfo[b * 2 + p_][0]
                    s = seg_id(b, t, p_)
                    a0 = int(base[s]) - c0
                    a1 = a0 + int(S_seg[s])
                    mc_base[s] = mc
                    for col in range(a0 // P, -(-a1 // P)):
                        pl.append((b, mc, col, p_))
                        mc += 1
            pieces[t] = pl
        group_pieces.append(pieces)
    ncols_total = mc

    # ---- per-core cell tensors
    core_tensors = []
    # per-cell mapping: global pos -> call-local (partition, buf col, mask col)
    call_cell0 = np.sort(np.array(
        [cinfo[k][0] for cinfo in calls for k in range(2 * nblocks)], np.int64))

    def cell_to_rowcol(pos, s_s):
        ci = np.searchsorted(call_cell0, pos, side="right") - 1
        local = pos - call_cell0[ci]
        col = local // P
        a0col = (base[s_s] - call_cell0[ci]) // P
        return (local % P).astype(np.int64), mc_base[s_s] + col - a0col

    for c in range(NCORES):
        s_c, pl_c, ld_c, d_c = per_core[c]
        order = np.argsort(s_c, kind="stable")
        s_s = s_c[order]
        cnt = counts[c]
        starts = np.zeros(nseg, np.int64)
        np.cumsum(cnt[:-1], out=starts[1:])
        rank = np.arange(s_s.shape[0], dtype=np.int64) - np.repeat(starts, cnt)
        pos = base[s_s] + rank
        prow, mcol = cell_to_rowcol(pos, s_s)

        idx_cells = np.zeros(total_cells, np.int16)          # pad -> pair 0
        dstrel = np.full((P, ncols_total + ntiles), -1.0, np.float32)
        edinv = np.zeros((P, ncols_total + ntiles), np.float32)
        idx_cells[pos] = pl_c[order].astype(np.int16)
        dstrel[prow, mcol] = (ld_c[order] % P).astype(np.float32)
        edinv[prow, mcol] = dinv[d_c[order]].astype(np.float32)

        idx16 = idx_cells.reshape(-1, 16).T                  # [16, cells/16]
        idx_w = np.tile(idx16, (8, 1)).copy()                # [128, cells/16]

        lo = c * shard_real
        hi = min(N, lo + shard_real)
        dvec = np.ones(shard_p, np.float32)
        dvec[pos_list[c]] = dinv[lo:hi]
        dinv_col = dvec.reshape(ntiles, P).T.astype(np.float32).copy()

        # self columns (one per tile, after gather cols)
        valid = np.zeros(shard_p, bool)
        valid[pos_list[c]] = True
        sd = np.full(shard_p, -1.0, np.float32)
        se = np.zeros(shard_p, np.float32)
        sd[valid] = (np.arange(shard_p) % P).astype(np.float32)[valid]
        se[valid] = dvec[valid]
        dstrel[:, ncols_total:] = sd.reshape(ntiles, P).T
        edinv[:, ncols_total:] = se.reshape(ntiles, P).T

        core_tensors.append(
            dict(idx=idx_w, dst_rel=np.ascontiguousarray(dstrel),
                 edinv=np.ascontiguousarray(edinv), dinv_col=dinv_col,
                 pos=pos_list[c])
        )

    sched = dict(
        shard_real=shard_real,
        shard_p=shard_p,
        blk_nodes=blk_nodes,
        nblocks=nblocks,
        ntiles=ntiles,
        tile_groups=tile_groups,
        calls=calls,
        group_pieces=group_pieces,
        ncols_total=ncols_total,
        total_cells=total_cells,
    )
    return sched, core_tensors


# ----------------------------------------------------------------------------
# bass builder
# ----------------------------------------------------------------------------
def _build_bass(sched):
    shard_p = sched["shard_p"]
    blk_nodes = sched["blk_nodes"]
    nblocks = sched["nblocks"]
    ntiles = sched["ntiles"]
    tile_groups = sched["tile_groups"]
    calls = sched["calls"]
    group_pieces = sched["group_pieces"]
    ncols_total = sched["ncols_total"]
    total_cells = sched["total_cells"]
    table_rows = NCORES * shard_p
    blk_pairs = blk_nodes // 2

    TSIM = bool(int(os.environ.get("GNN_TSIM", "0")))
    nc = bacc.Bacc("TRN2", target_bir_lowering=False,
                   num_devices=1 if TSIM else NCORES,
                   dynamic_dma_scratch_size=int(os.environ.get("GNN_SCRATCH", "65536")))

    emb_in = nc.dram_tensor("emb_fm", [H, shard_p], F16, kind="ExternalInput")
    idx_in = nc.dram_tensor("idx", [P, total_cells // 16], mybir.dt.int16,
                            kind="ExternalInput")
    dst_rel_in = nc.dram_tensor("dst_rel", [P, ncols_total + ntiles], F32,
                                kind="ExternalInput")
    edinv_in = nc.dram_tensor("edinv", [P, ncols_total + ntiles], F32,
                              kind="ExternalInput")
    dinv_col_in = nc.dram_tensor("dinv_col", [P, ntiles], F32, kind="ExternalInput")
    iota_in = nc.dram_tensor("iota", [P, P], F16, kind="ExternalInput")
    w_ins = [nc.dram_tensor(f"W{l+1}", [H, H], F16, kind="ExternalInput")
             for l in range(3)]
    b_ins = [nc.dram_tensor(f"b{l+1}", [H, 1], F32, kind="ExternalInput")
             for l in range(3)]
    out_fm = nc.dram_tensor("out_fm", [H, shard_p], F16, kind="ExternalOutput")

    NL = int(os.environ.get("GNN_NLAYERS", "3"))
    ACT_EVAC = os.environ.get("GNN_ACT", "0") == "1"
    MGB = int(os.environ.get("GNN_MGB", "3"))

    with tile.TileContext(nc) as tc:
        with (
            tc.tile_pool(name="persist", bufs=1) as persist,
            tc.tile_pool(name="msgs", bufs=MGB) as msgs_pool,
            tc.tile_pool(name="masks", bufs=int(os.environ.get("GNN_MKB", "32"))) as mask_pool,
            tc.tile_pool(name="ps_agg", bufs=int(os.environ.get("GNN_PSB", "6")), space="PSUM") as ps_agg,
            tc.tile_pool(name="ps_tb", bufs=2, space="PSUM") as ps_tb,
            tc.tile_pool(name="dram", bufs=1, space="DRAM") as dram,
        ):
            # ---- persistent SBUF ----
            x_fm = persist.tile([H, shard_p], F16)
            yshard = persist.tile([P, ntiles, H], F16)
            idx_sb = persist.tile([P, total_cells // 16], mybir.dt.int16)
            dst_rel = persist.tile([P, ncols_total + ntiles], F32)
            edinv = persist.tile([P, ncols_total + ntiles], F32)
            dinv_col = persist.tile([P, ntiles], F32)
            iota_sb = persist.tile([P, P], F16)
            w_sb = [persist.tile([H, H], F16, name=f"w{l}") for l in range(3)]
            b_sb = [persist.tile([H, 1], F32, name=f"b{l}") for l in range(3)]

            nc.sync.dma_start(out=x_fm[:], in_=emb_in[:])
            nc.sync.dma_start(out=idx_sb[:], in_=idx_in[:])
            nc.sync.dma_start(out=dst_rel[:], in_=dst_rel_in[:])
            nc.sync.dma_start(out=edinv[:], in_=edinv_in[:])
            nc.sync.dma_start(out=dinv_col[:], in_=dinv_col_in[:])
            nc.sync.dma_start(out=iota_sb[:], in_=iota_in[:])
            for l in range(3):
                nc.sync.dma_start(out=w_sb[l][:], in_=w_ins[l][:])
                nc.sync.dma_start(out=b_sb[l][:], in_=b_ins[l][:])

            ag_in = [dram.tile([shard_p, H], F16, name=f"agin{i}") for i in range(3)]
            tables = [dram.tile([table_rows, H], F16, addr_space="Shared",
                                name=f"table{i}") for i in range(3)]

            POOLMASK = int(os.environ.get("GNN_POOLMASK", "0"))
            mask_ctr = [0]

            def mk_mask(col):
                mask = mask_pool.tile([P, P], F16, tag="mask")
                eng = nc.gpsimd if (POOLMASK and mask_ctr[0] % POOLMASK == 0) \
                    else nc.vector
                mask_ctr[0] += 1
                eng.tensor_scalar(
                    mask[:], iota_sb[:],
                    dst_rel[:, col:col + 1],
                    edinv[:, col:col + 1],
                    mybir.AluOpType.is_equal,
                    mybir.AluOpType.mult,
                )
                return mask

            def table_tile(l, t):
                pt = ps_tb.tile([P, H], F32, space="PSUM", tag="pt")
                nc.tensor.matmul(
                    out=pt[:],
                    lhsT=x_fm[:, t * P:(t + 1) * P],
                    rhs=w_sb[l][:],
                    start=True, stop=True,
                )
                if ACT_EVAC:
                    nc.scalar.mul(yshard[:, t, :], pt[:], dinv_col[:, t:t + 1])
                else:
                    nc.vector.tensor_scalar_mul(
                        yshard[:, t, :], pt[:], dinv_col[:, t:t + 1])

            def gi_write(l, t0, t1):
                nc.sync.dma_start(
                    out=ag_in[l][t0 * P:t1 * P, :].rearrange(
                        "(a p) h -> p a h", p=P),
                    in_=yshard[:, t0:t1, :],
                )

            def distribute(l):
                if TSIM:
                    nc.sync.dma_start(out=tables[l][0:shard_p, :],
                                      in_=ag_in[l][:])
                else:
                    nc.gpsimd.collective_compute(
                        "AllGather",
                        mybir.AluOpType.bypass,
                        replica_groups=[list(range(NCORES))],
                        ins=[ag_in[l][:].opt()],
                        outs=[tables[l][: NCORES * shard_p, :].opt()],
                    )

            # ---- prologue: layer-0 table from the embeddings ----
            for t in range(ntiles):
                table_tile(0, t)
            for t0 in range(0, ntiles, 14):
                gi_write(0, t0, min(ntiles, t0 + 14))
            distribute(0)

            # emit groups largest-first so the smallest group drains the
            # pipeline at each layer boundary (shortest AllGather stall)
            emit_order = sorted(
                range(len(tile_groups)),
                key=lambda g: -max(c[1] for c in calls[g]))
            for l in range(NL):
                tb = tables[l]
                # ---- aggregation (tile groups; pair-view gather); the next
                # layer's table phase is interleaved per tile so only the
                # AllGather remains at the layer boundary ----
                tb4 = tb[:].rearrange("(r four) h -> r (four h)", four=4)
                for gidx in emit_order:
                    t0, t1 = tile_groups[gidx]
                    cinfo = calls[gidx]
                    bufs = []
                    for k in range(2 * nblocks):
                        cell0, ncells = cinfo[k]
                        if ncells == 0:
                            bufs.append(None)
                            continue
                        ncols = -(-ncells // P)
                        buf = msgs_pool.tile([P, ncols, H], F16,
                                             tag=f"msgs{k}", bufs=MGB)
                        _narrow_gather(
                            nc.gpsimd, buf[:],
                            tb4[:, k * H:(k + 1) * H],
                            idx_sb[:, cell0 // 16: (cell0 + ncells) // 16],
                            ncells, H, 4 * H,
                        )
                        bufs.append(buf)
                    for t in range(t0, t1):
                        psum = ps_agg.tile([H, P], F32, space="PSUM",
                                           tag="pagg")
                        pl = group_pieces[gidx][t]
                        mi = 0
                        for (b, mcol, bufcol, p_) in pl:
                            buf = bufs[b * 2 + p_]
                            mask = mk_mask(mcol)
                            nc.tensor.matmul(
                                out=psum[:],
                                lhsT=buf[:, bufcol, :],
                                rhs=mask[:],
                                start=(mi == 0), stop=False,
                            )
                            mi += 1
                        mask = mk_mask(ncols_total + t)
                        nc.tensor.matmul(
                            out=psum[:],
                            lhsT=yshard[:, t, :],
                            rhs=mask[:],
                            start=(mi == 0), stop=True,
                        )
                        if ACT_EVAC:
                            nc.scalar.add(x_fm[:, t * P:(t + 1) * P],
                                          psum[:], b_sb[l][:])
                        else:
                            nc.vector.tensor_scalar(
                                x_fm[:, t * P:(t + 1) * P], psum[:],
                                b_sb[l][:], None, mybir.AluOpType.add,
                            )
                        if l + 1 < NL:
                            table_tile(l + 1, t)
                    if l + 1 < NL:
                        gi_write(l + 1, t0, t1)
                    else:
                        nc.sync.dma_start(
                            out=out_fm[:, t0 * P:t1 * P],
                            in_=x_fm[:, t0 * P:t1 * P],
                        )
                if l + 1 < NL:
                    distribute(l + 1)

    nc.compile()
    return nc


_CACHE = {}


def _prep_inputs(embeddings, edge_index, W1, b1, W2, b2, W3, b3):
    embeddings = np.ascontiguousarray(np.asarray(embeddings, dtype=np.float32))
    edge_index = np.asarray(edge_index)
    N = embeddings.shape[0]

    sched, core_tensors = _build_schedule(edge_index, N)
    shard_real, shard_p = sched["shard_real"], sched["shard_p"]

    key = (N, edge_index.shape[1], sched["total_cells"],
           tuple(sorted((k, v) for k, v in os.environ.items()
                        if k.startswith("GNN_"))))
    if key not in _CACHE:
        _CACHE[key] = _build_bass(sched)
    nc = _CACHE[key]

    iota = np.tile(np.arange(P, dtype=np.float16), (P, 1)).copy()
    ws = [np.asarray(W, np.float16) for W in (W1, W2, W3)]
    bs = [np.asarray(b, np.float32).reshape(H, 1) for b in (b1, b2, b3)]

    in_maps = []
    for c in range(NCORES):
        lo = c * shard_real
        hi = min(N, lo + shard_real)
        ct = core_tensors[c]
        emb_fm = np.zeros((H, shard_p), np.float16)
        emb_fm[:, ct["pos"]] = embeddings[lo:hi].T
        in_maps.append(dict(
            emb_fm=emb_fm, idx=ct["idx"], dst_rel=ct["dst_rel"],
            edinv=ct["edinv"], dinv_col=ct["dinv_col"], iota=iota,
            W1=ws[0], W2=ws[1], W3=ws[2], b1=bs[0], b2=bs[1], b3=bs[2],
        ))
    return nc, in_maps, sched, core_tensors


def kernel(embeddings, edge_index, W1, b1, W2, b2, W3, b3):
    N = np.asarray(embeddings).shape[0]
    nc, in_maps, sched, core_tensors = _prep_inputs(
        embeddings, edge_index, W1, b1, W2, b2, W3, b3)
    shard_real = sched["shard_real"]

    res = run_bass_kernel_spmd(nc, in_maps, core_ids=list(range(NCORES)))
    out = np.empty((N, H), np.float32)
    for c in range(NCORES):
        lo = c * shard_real
        hi = min(N, lo + shard_real)
        out[lo:hi] = res.results[c]["out_fm"].T[
            core_tensors[c]["pos"]].astype(np.float32)
    return out


def prepare(embeddings, edge_index, W1, b1, W2, b2, W3, b3):
    """Build (nc, in_maps) once for repeated benchmarking."""
    nc, in_maps, sched, _ct = _prep_inputs(
        embeddings, edge_index, W1, b1, W2, b2, W3, b3)
    return nc, in_maps, sched

